# revision 22
# baseline (speedup 1.0000x reference)
"""FFM (field-aware factorization machine) forward kernel for 8 TRN2 NeuronCores.

y[b] = x[b] @ w_lin + b_lin + sum_{i<j} Wu[i,j] x[b,i] x[b,j]
with Wu = triu(Wmat, 1), Wmat[i,j] = <v[i, field[j]], v[j, field[i]]>.

Strategy (v5):
  - Host: build Wmat from (v, field_idx) [tiny], symmetrize
    S = (Wu + Wu^T)/2, eigendecompose S = Q diag(lam) Q^T. Shift the
    spectrum by c = -lam_min so mu = lam + c >= 0, fold sqrt(mu) into the
    eigenvectors: Q' = Q diag(sqrt(mu)). Then
      x^T Wu x = sum_n mu_n (x . q_n)^2 - c ||x||^2
    and the -c||x||^2 correction joins the (host-computed) linear part.
  - Device (data-parallel over batch, 8 cores): per 128-sample chunk,
    PE computes z = x_chunk^T Q' with batch on PSUM partitions and the
    eigen index on the free dim (two bf16 matmuls, contraction 256).
    The per-sample reduction sum_n z_n^2 runs along the FREE dim, with
    whole PSUM GROUPS assigned to one engine (group-level split avoids
    ACT/DVE hammering the same PSUM banks):
      * ACT groups: ScalarE Square(psum)+accum_out per chunk.
      * DVE groups: VectorE bn_stats per chunk straight from PSUM;
        sum z^2 is reconstructed from the even/odd (count,mean,
        count*var) stats in one batched 5-op fixup at rep end:
          sum z^2 = cv_e + 128 me^2 + cv_o + 128 mo^2.
        This avoids the PSUM->SBUF copy entirely (PSUM may be read
        once per DVE instr; bn_stats needs it only once).
  - x ships as bf16 pre-transposed in 8 slab DMAs on the SP HWDGE ring
    with a deep (8-buf) SBUF pipeline; slab layout keeps each partition
    row contiguous in DRAM.
  - y columns are engine-ordered (ACT cols first, then DVE cols);
    the host inverts the permutation.
"""

import numpy as np

_LDW_OPT = {"on": False}


def _install_walrus_patch():
    """Allow flipping walrus --enable-ldw-opt at NEFF-compile time."""
    from concourse import bass_utils
    if getattr(bass_utils, "_ant_ldw_patched", False):
        return
    orig = bass_utils.run_command

    def patched(cmd, *a, **kw):
        if _LDW_OPT["on"] and isinstance(cmd, list):
            cmd = [c.replace("--enable-ldw-opt=false", "--enable-ldw-opt=true")
                   if isinstance(c, str) else c for c in cmd]
        return orig(cmd, *a, **kw)

    bass_utils.run_command = patched
    bass_utils._ant_ldw_patched = True


_B, _N = 65536, 256
_NCORES = 8
_BS = _B // _NCORES   # 8192 samples per core
_NCH = _BS // 128     # 64 batch chunks per core
_DCH = 1024           # DMA slab columns

_compiled_nc = {}


def _assign(act_chunks, gsz, nch=_NCH):
    """Group-level engine assignment: whole groups go to ACT until
    act_chunks is covered (rounded to groups), interleaved evenly.

    Returns (n_act_of[g], act_order, dve_order)."""
    n_groups = nch // gsz
    n_act_groups = min(n_groups, round(act_chunks / gsz))
    # spread ACT groups evenly among all groups
    is_act = [False] * n_groups
    if n_act_groups:
        for i in range(n_act_groups):
            is_act[(i * n_groups) // n_act_groups] = True
    n_act_of = [gsz if a else 0 for a in is_act]
    act_order, dve_order = [], []
    for g in range(n_groups):
        for j in range(gsz):
            c = g * gsz + j
            (act_order if j < n_act_of[g] else dve_order).append(c)
    return n_act_of, act_order, dve_order


def _build_nc(reps=1, mode="full", act_chunks=24, xin_bufs=8,
              gsz=2, pz_bufs=8, dch=1024, dma_alt=False, xlayout="slab",
              ldw_opt=0, salt=0, scr_mode="inplace", fix_eng="vector", obufs=2):
    """v5 kernel; see module docstring. Probe modes:
      full    - everything
      dmaonly - x DMAs only
      noxdma  - MMs+reducers from undisturbed SBUF (no x DMA)
      nored   - DMAs + MMs, no reducers
      samew   - DMAs + MMs with a CONSTANT stationary operand (junk
                math; probes whether unchanged weights skip/hide LDW)
      samewnx - samew without x DMAs (pure-PE constant-weight probe)
      mmonly  - real MM structure without x DMAs or reducers
    """
    from concourse import bacc, mybir, tile

    f32 = mybir.dt.float32
    bf16 = mybir.dt.bfloat16
    Act = mybir.ActivationFunctionType
    Alu = mybir.AluOpType

    assert _BS % dch == 0
    nslab = _BS // dch
    n_groups = _NCH // gsz
    n_act_of, act_order, dve_order = _assign(act_chunks, gsz)
    n_act_total = len(act_order)
    n_dve_total = len(dve_order)

    _install_walrus_patch()
    _LDW_OPT["on"] = bool(ldw_opt)

    nc = bacc.Bacc("TRN2", target_bir_lowering=False, debug=False)

    # salt defeats the NEFF cache when only compile flags change
    if salt:
        nc.dram_tensor(f"salt{salt}", [1, 1], mybir.dt.float32,
                       kind="ExternalOutput")

    # x pre-transposed. Two DRAM layouts:
    #   row:  [2, 128, _BS] row-major; a slab reads a column range of
    #         every (blk, p) row -> 2*128 descriptors of dch*2B, 16KB
    #         DRAM stride between descriptors.
    #   slab: [nslab, 128, 2, dch]; slab s partition p is contiguous
    #         (2*dch*2B descriptors).
    if xlayout == "row":
        xt = nc.dram_tensor("xt", [2, 128, _BS], bf16,
                            kind="ExternalInput").ap()
    else:
        xt = nc.dram_tensor("xt", [nslab, 128, 2, dch], bf16,
                            kind="ExternalInput").ap()

    def x_slab_src(s):
        if xlayout == "row":
            return xt[:, :, s * dch:(s + 1) * dch].transpose([1, 0, 2])
        return xt[s]
    qp = nc.dram_tensor("qp", [2, 128, _N], bf16, kind="ExternalInput").ap()
    # y[p, k] = sum_n z^2 for engine-ordered chunk k (ACT cols, then DVE)
    y = nc.dram_tensor("y", [128, _NCH], f32, kind="ExternalOutput").ap()

    dma_engines = [nc.sync, nc.scalar] if dma_alt else [nc.sync]

    with tile.TileContext(nc) as tc:
        with (
            tc.tile_pool(name="const", bufs=1) as cpool,
            tc.tile_pool(name="xin", bufs=xin_bufs) as xpool,
            tc.tile_pool(name="yout", bufs=obufs) as ypool,
            tc.tile_pool(name="stat", bufs=obufs) as stpool,
            tc.tile_pool(name="scr", bufs=4) as spool,
            tc.tile_pool(name="fix", bufs=2) as fpool,
            tc.tile_pool(name="pz", bufs=pz_bufs, space="PSUM") as pzpool,
        ):
            q_sb = cpool.tile([128, 2, _N], bf16)
            nc.sync.dma_start(q_sb[:], qp[:, :, :].transpose([1, 0, 2]))
            if mode in ("noxdma", "samewnx", "mmonly"):
                xfix = cpool.tile([128, 2, dch], bf16)
                nc.sync.dma_start(xfix[:], x_slab_src(0))

            scr_fix = None
            if scr_mode == "fixed":
                scr_fix = cpool.tile([128, _N], bf16)

            def emit_group(g, pz, y_sb, st_sb, cursors):
                n_act = n_act_of[g]
                for j in range(n_act):
                    if scr_mode == "inplace":
                        scr = pz[:, j, :]
                    elif scr_mode == "fixed":
                        scr = scr_fix[:]
                    else:
                        scr_t = spool.tile([128, _N], bf16, tag="s")
                        scr = scr_t[:]
                    nc.scalar.activation(
                        scr, pz[:, j, :], Act.Square,
                        accum_out=y_sb[:, cursors[0]:cursors[0] + 1])
                    cursors[0] += 1
                for j in range(n_act, gsz):
                    nc.vector.bn_stats(st_sb[:, cursors[1], :], pz[:, j, :])
                    cursors[1] += 1

            for _rep in range(reps):
                y_sb = ypool.tile([128, _NCH], f32, tag="y")
                st_sb = stpool.tile([128, max(n_dve_total, 1), 6], f32,
                                    tag="st")
                cursors = [0, 0]   # [next ACT y col, next stats slot]
                c = 0
                for s in range(nslab):
                    if mode in ("noxdma", "samewnx", "mmonly"):
                        x_sb = xfix
                    else:
                        x_sb = xpool.tile([128, 2, dch], bf16, tag="x")
                        eng = dma_engines[s % len(dma_engines)]
                        eng.dma_start(x_sb[:], x_slab_src(s))
                    for k in range(dch // 128):
                        j = c % gsz
                        if j == 0:
                            pz = pzpool.tile([128, gsz, _N], f32, tag="pz")
                        if mode in ("samew", "samewnx"):
                            nc.tensor.matmul(pz[:, j, :],
                                             q_sb[:, 0, 0:128],
                                             x_sb[:, 0, :].rearrange(
                                                 "p (a b) -> p a b", b=256)
                                             [:, k % (dch // 256), :],
                                             start=True, stop=True)
                            c += 1
                            continue
                        nc.tensor.matmul(pz[:, j, :],
                                         x_sb[:, 0, k * 128:(k + 1) * 128],
                                         q_sb[:, 0, :], start=True, stop=False)
                        nc.tensor.matmul(pz[:, j, :],
                                         x_sb[:, 1, k * 128:(k + 1) * 128],
                                         q_sb[:, 1, :], start=False, stop=True)
                        c += 1
                        if mode in ("nored", "samew", "samewnx", "mmonly"):
                            continue
                        if j == gsz - 1:
                            emit_group(c // gsz - 1, pz, y_sb, st_sb, cursors)
                if mode == "full":
                    # fixup: y_dve = cv_e + cv_o + 128*(me^2 + mo^2)
                    nd = n_dve_total
                    if nd:
                        feng = getattr(nc, fix_eng)
                        u = fpool.tile([128, nd, 4], f32, tag="u")
                        feng.tensor_tensor(
                            out=u[:, :, 0], in0=st_sb[:, :, 2],
                            in1=st_sb[:, :, 5], op=Alu.add)
                        feng.tensor_tensor(
                            out=u[:, :, 1], in0=st_sb[:, :, 1],
                            in1=st_sb[:, :, 1], op=Alu.mult)
                        feng.tensor_tensor(
                            out=u[:, :, 2], in0=st_sb[:, :, 4],
                            in1=st_sb[:, :, 4], op=Alu.mult)
                        feng.tensor_tensor(
                            out=u[:, :, 3], in0=u[:, :, 1],
                            in1=u[:, :, 2], op=Alu.add)
                        nc.vector.scalar_tensor_tensor(
                            out=y_sb[:, n_act_total:n_act_total + nd],
                            in0=u[:, :, 3], scalar=128.0, in1=u[:, :, 0],
                            op0=Alu.mult, op1=Alu.add)
                    nc.sync.dma_start(y[:, :], y_sb[:])

    nc.compile()
    return nc


def _get_nc(reps=1, **kw):
    key = (reps,) + tuple(sorted(kw.items()))
    if key not in _compiled_nc:
        _compiled_nc[key] = _build_nc(reps, **kw)
    return _compiled_nc[key]


def _to_bf16(a):
    import ml_dtypes

    return np.ascontiguousarray(a).astype(ml_dtypes.bfloat16)


def _host_prep(x, w_lin, b_lin, v, field_idx, dch=1024, xlayout="slab"):
    """Host-side tiny-param preprocessing + sharding. Returns (in_maps, lin)."""
    x = np.asarray(x, dtype=np.float32)
    w_lin = np.asarray(w_lin, dtype=np.float64)
    b_lin = np.asarray(b_lin, dtype=np.float64)
    v = np.asarray(v, dtype=np.float64)
    field_idx = np.asarray(field_idx, dtype=np.int64)

    # Wmat[i, j] = <v[i, field[j]], v[j, field[i]]>
    A = v[:, field_idx, :]                       # [N, N, K]
    Wmat = np.einsum('ijk,jik->ij', A, A)        # [N, N]
    Wu = np.triu(Wmat, 1)
    S = (Wu + Wu.T) * 0.5
    lam, Q = np.linalg.eigh(S)                   # ascending eigenvalues
    c = max(0.0, -lam[0])
    mu = np.clip(lam + c, 0.0, None)
    # [N, N] column-scaled, contraction split into 2 blocks of 128 rows
    Qp = _to_bf16(Q * np.sqrt(mu)[None, :]).reshape(2, 128, _N)

    nslab = _BS // dch
    x64 = x.astype(np.float64)
    if xlayout == "row":
        # [core, blk, p, col] = x^T split into 2 contraction blocks
        xts = x.reshape(_NCORES, _BS, _N).transpose(0, 2, 1)  # [8, N, BS]
        xts = _to_bf16(xts).reshape(_NCORES, 2, 128, _BS)
    else:
        # [core, slab, p, blk, col]: (blk, col) contiguous per partition
        xts = _to_bf16(x).reshape(_NCORES, nslab, dch, 2, 128)
        xts = np.ascontiguousarray(xts.transpose(0, 1, 4, 3, 2))

    in_maps = [{"xt": xts[i], "qp": Qp} for i in range(_NCORES)]
    # linear part and the -c||x||^2 spectrum-shift correction, both host-side
    lin = x64 @ w_lin + b_lin[0] - c * np.einsum('bi,bi->b', x64, x64)
    return in_maps, lin


def _y_perm(act_chunks=24, gsz=2):
    """Column permutation: y_sb engine-order col -> chunk id."""
    _, act_order, dve_order = _assign(act_chunks, gsz)
    return np.array(act_order + dve_order)


def _run_device(in_maps, trace=False, reps=1, **kw):
    from concourse.bass_utils import run_bass_kernel_spmd

    nc = _get_nc(reps, **kw)
    res = run_bass_kernel_spmd(
        nc, in_maps, core_ids=list(range(_NCORES)), trace=trace
    )
    perm = _y_perm(kw.get("act_chunks", 24), kw.get("gsz", 2))
    inv = np.empty_like(perm)
    inv[perm] = np.arange(len(perm))
    # y[p, k] (engine order) -> chunk order -> batch order
    yq = np.concatenate(
        [np.asarray(res.results[i]["y"], dtype=np.float64)[:, inv]
         .T.reshape(-1)
         for i in range(_NCORES)]
    )
    return yq, res


def kernel(x, w_lin, b_lin, v, field_idx):
    in_maps, lin = _host_prep(x, w_lin, b_lin, v, field_idx)
    yq, _ = _run_device(in_maps, trace=False)
    return (lin + yq).astype(np.float32)[:, None]


# revision 27
# speedup vs baseline: 1.1261x; 1.1261x over previous
"""FFM (field-aware factorization machine) forward kernel for 8 TRN2 NeuronCores.

y[b] = x[b] @ w_lin + b_lin + sum_{i<j} Wu[i,j] x[b,i] x[b,j]
with Wu = triu(Wmat, 1), Wmat[i,j] = <v[i, field[j]], v[j, field[i]]>.

Strategy (v6):
  - Host: build Wmat from (v, field_idx) [tiny], symmetrize
    S = (Wu + Wu^T)/2, eigendecompose S = Q diag(lam) Q^T. MEDIAN shift:
    c = -median(lam) makes exactly 128 shifted eigenvalues positive and
    128 negative with |c| tiny (no cancellation amplification), so
      x^T Wu x = sum_n sign(mu_n) (x . q'_n)^2 - c ||x||^2,
    q' = q sqrt(|mu|). The -c||x||^2 correction joins the host linear
    part. The tiny shift keeps quantization error low enough that the
    SECOND contraction block of x ships as fp8e3 (e3m4) - 25% less DMA
    (rel err 0.0138 vs the 0.02 budget, deterministic inputs).
  - Device (data-parallel over batch, 8 cores): per 128-sample chunk,
    PE computes z = x_chunk^T Q' with batch on PSUM partitions and the
    eigen index on the free dim (bf16 block-0 + fp8e3 block-1 matmuls,
    mixed-dtype verified exact). Reduction sum_n sign_n z_n^2 runs on
    the FREE dim, whole PSUM GROUPS per engine:
      * ACT groups: rhs = Q'_act ([pos block | neg block] column order);
        ScalarE Square(psum)+accum_out per 128-col segment -> (pos,neg)
        y-col pair; host subtracts.
      * DVE groups: rhs = Q'_dve (pos at EVEN, neg at ODD columns);
        VectorE bn_stats per chunk straight from PSUM - its even/odd
        (count,mean,count*var) stats give the SIGNED sum in one batched
        5-op fixup:  y = (cv_e + 128 me^2) - (cv_o + 128 mo^2).
        No PSUM->SBUF copy (PSUM read once per DVE instr).
  - x ships pre-transposed in 8 slab DMA pairs on the SP HWDGE ring
    with a deep (8-buf) SBUF pipeline.
  - y columns are engine-ordered (ACT (pos,neg) pairs first, then DVE
    cols); the host subtracts and inverts the permutation.
"""

import numpy as np

_LDW_OPT = {"on": False}


def _install_walrus_patch():
    """Allow flipping walrus --enable-ldw-opt at NEFF-compile time."""
    from concourse import bass_utils
    if getattr(bass_utils, "_ant_ldw_patched", False):
        return
    orig = bass_utils.run_command

    def patched(cmd, *a, **kw):
        if _LDW_OPT["on"] and isinstance(cmd, list):
            cmd = [c.replace("--enable-ldw-opt=false", "--enable-ldw-opt=true")
                   if isinstance(c, str) else c for c in cmd]
        return orig(cmd, *a, **kw)

    bass_utils.run_command = patched
    bass_utils._ant_ldw_patched = True


_B, _N = 65536, 256
_NCORES = 8
_BS = _B // _NCORES   # 8192 samples per core
_NCH = _BS // 128     # 64 batch chunks per core
_DCH = 1024           # DMA slab columns

_compiled_nc = {}


def _assign(act_chunks, gsz, nch=_NCH):
    """Group-level engine assignment: whole groups go to ACT until
    act_chunks is covered (rounded to groups), interleaved evenly.

    Returns (n_act_of[g], act_order, dve_order)."""
    n_groups = nch // gsz
    n_act_groups = min(n_groups, round(act_chunks / gsz))
    # spread ACT groups evenly among all groups
    is_act = [False] * n_groups
    if n_act_groups:
        for i in range(n_act_groups):
            is_act[(i * n_groups) // n_act_groups] = True
    n_act_of = [gsz if a else 0 for a in is_act]
    act_order, dve_order = [], []
    for g in range(n_groups):
        for j in range(gsz):
            c = g * gsz + j
            (act_order if j < n_act_of[g] else dve_order).append(c)
    return n_act_of, act_order, dve_order


def _build_nc(reps=1, mode="full", act_chunks=18, xin_bufs=8,
              gsz=2, pz_bufs=8, dch=1024, dma_alt=False, xlayout="slab",
              ldw_opt=0, salt=0, scr_mode="inplace", fix_eng="vector", obufs=2,
              x8=1):
    """v5 kernel; see module docstring. Probe modes:
      full    - everything
      dmaonly - x DMAs only
      noxdma  - MMs+reducers from undisturbed SBUF (no x DMA)
      nored   - DMAs + MMs, no reducers
      samew   - DMAs + MMs with a CONSTANT stationary operand (junk
                math; probes whether unchanged weights skip/hide LDW)
      samewnx - samew without x DMAs (pure-PE constant-weight probe)
      mmonly  - real MM structure without x DMAs or reducers
    """
    from concourse import bacc, mybir, tile

    f32 = mybir.dt.float32
    bf16 = mybir.dt.bfloat16
    Act = mybir.ActivationFunctionType
    Alu = mybir.AluOpType

    assert _BS % dch == 0
    nslab = _BS // dch
    n_groups = _NCH // gsz
    n_act_of, act_order, dve_order = _assign(act_chunks, gsz)
    n_act_total = len(act_order)
    n_dve_total = len(dve_order)

    _install_walrus_patch()
    _LDW_OPT["on"] = bool(ldw_opt)

    nc = bacc.Bacc("TRN2", target_bir_lowering=False, debug=False)

    # salt defeats the NEFF cache when only compile flags change
    if salt:
        nc.dram_tensor(f"salt{salt}", [1, 1], mybir.dt.float32,
                       kind="ExternalOutput")

    # x pre-transposed. Two DRAM layouts:
    #   row:  [2, 128, _BS] row-major; a slab reads a column range of
    #         every (blk, p) row -> 2*128 descriptors of dch*2B, 16KB
    #         DRAM stride between descriptors.
    #   slab: [nslab, 128, 2, dch]; slab s partition p is contiguous
    #         (2*dch*2B descriptors).
    fp8 = mybir.dt.float8e3
    xte = None
    if x8:
        assert xlayout == "slab"
        xt = nc.dram_tensor("xt", [nslab, 128, 1, dch], bf16,
                            kind="ExternalInput").ap()
        xte = nc.dram_tensor("xte", [nslab, 128, 1, dch], fp8,
                             kind="ExternalInput").ap()
    elif xlayout == "row":
        xt = nc.dram_tensor("xt", [2, 128, _BS], bf16,
                            kind="ExternalInput").ap()
    else:
        xt = nc.dram_tensor("xt", [nslab, 128, 2, dch], bf16,
                            kind="ExternalInput").ap()

    def x_slab_src(s):
        if xlayout == "row":
            return xt[:, :, s * dch:(s + 1) * dch].transpose([1, 0, 2])
        return xt[s]
    qp = nc.dram_tensor("qp", [2, 2, 128, _N], bf16,
                        kind="ExternalInput").ap()
    # y cols: per ACT chunk a (pos, neg) pair, then one col per DVE chunk
    y = nc.dram_tensor("y", [128, _NCH + n_act_total], f32,
                       kind="ExternalOutput").ap()

    dma_engines = [nc.sync, nc.scalar] if dma_alt else [nc.sync]

    with tile.TileContext(nc) as tc:
        with (
            tc.tile_pool(name="const", bufs=1) as cpool,
            tc.tile_pool(name="xin", bufs=xin_bufs) as xpool,
            tc.tile_pool(name="yout", bufs=obufs) as ypool,
            tc.tile_pool(name="stat", bufs=obufs) as stpool,
            tc.tile_pool(name="scr", bufs=4) as spool,
            tc.tile_pool(name="fix", bufs=2) as fpool,
            tc.tile_pool(name="pz", bufs=pz_bufs, space="PSUM") as pzpool,
        ):
            q_sb = cpool.tile([128, 2, 2, _N], bf16)
            nc.sync.dma_start(q_sb[:], qp.transpose([2, 0, 1, 3]))
            if mode in ("noxdma", "samewnx", "mmonly"):
                xfix = cpool.tile([128, 2, dch], bf16)
                nc.sync.dma_start(xfix[:], x_slab_src(0))

            scr_fix = None
            if scr_mode == "fixed":
                scr_fix = cpool.tile([128, _N], bf16)

            def emit_group(g, pz, y_sb, st_sb, cursors):
                n_act = n_act_of[g]
                for j in range(n_act):
                    for seg in range(2):
                        sl = pz[:, j, seg * 128:(seg + 1) * 128]
                        if scr_mode == "inplace":
                            scr = sl
                        else:
                            scr_t = spool.tile([128, 128], bf16, tag="s")
                            scr = scr_t[:]
                        nc.scalar.activation(
                            scr, sl, Act.Square,
                            accum_out=y_sb[:, cursors[0]:cursors[0] + 1])
                        cursors[0] += 1
                for j in range(n_act, gsz):
                    nc.vector.bn_stats(st_sb[:, cursors[1], :], pz[:, j, :])
                    cursors[1] += 1

            for _rep in range(reps):
                y_sb = ypool.tile([128, _NCH + n_act_total], f32, tag="y")
                st_sb = stpool.tile([128, max(n_dve_total, 1), 6], f32,
                                    tag="st")
                cursors = [0, 0]   # [next ACT y col, next stats slot]
                c = 0
                for s in range(nslab):
                    if mode in ("noxdma", "samewnx", "mmonly"):
                        x_sb = xfix
                        xe_sb = None
                    elif x8:
                        x_sb = xpool.tile([128, 1, dch], bf16, tag="x")
                        xe_sb = xpool.tile([128, 1, dch], fp8, tag="xe")
                        eng = dma_engines[s % len(dma_engines)]
                        eng.dma_start(x_sb[:], xt[s])
                        eng.dma_start(xe_sb[:], xte[s])
                    else:
                        x_sb = xpool.tile([128, 2, dch], bf16, tag="x")
                        xe_sb = None
                        eng = dma_engines[s % len(dma_engines)]
                        eng.dma_start(x_sb[:], x_slab_src(s))
                    for k in range(dch // 128):
                        j = c % gsz
                        if j == 0:
                            pz = pzpool.tile([128, gsz, _N], f32, tag="pz")
                        if mode in ("samew", "samewnx"):
                            nc.tensor.matmul(pz[:, j, :],
                                             q_sb[:, 0, 0:128],
                                             x_sb[:, 0, :].rearrange(
                                                 "p (a b) -> p a b", b=256)
                                             [:, k % (dch // 256), :],
                                             start=True, stop=True)
                            c += 1
                            continue
                        kind = 0 if n_act_of[c // gsz] else 1
                        nc.tensor.matmul(pz[:, j, :],
                                         x_sb[:, 0, k * 128:(k + 1) * 128],
                                         q_sb[:, kind, 0, :],
                                         start=True, stop=False)
                        xb1 = xe_sb if x8 else x_sb
                        nc.tensor.matmul(pz[:, j, :],
                                         xb1[:, 0 if x8 else 1,
                                             k * 128:(k + 1) * 128],
                                         q_sb[:, kind, 1, :],
                                         start=False, stop=True)
                        c += 1
                        if mode in ("nored", "samew", "samewnx", "mmonly"):
                            continue
                        if j == gsz - 1:
                            emit_group(c // gsz - 1, pz, y_sb, st_sb, cursors)
                if mode == "full":
                    # fixup: y_dve = cv_e + cv_o + 128*(me^2 + mo^2)
                    nd = n_dve_total
                    if nd:
                        feng = getattr(nc, fix_eng)
                        u = fpool.tile([128, nd, 4], f32, tag="u")
                        feng.tensor_tensor(
                            out=u[:, :, 0], in0=st_sb[:, :, 2],
                            in1=st_sb[:, :, 5], op=Alu.subtract)
                        feng.tensor_tensor(
                            out=u[:, :, 1], in0=st_sb[:, :, 1],
                            in1=st_sb[:, :, 1], op=Alu.mult)
                        feng.tensor_tensor(
                            out=u[:, :, 2], in0=st_sb[:, :, 4],
                            in1=st_sb[:, :, 4], op=Alu.mult)
                        feng.tensor_tensor(
                            out=u[:, :, 3], in0=u[:, :, 1],
                            in1=u[:, :, 2], op=Alu.subtract)
                        nc.vector.scalar_tensor_tensor(
                            out=y_sb[:, 2 * n_act_total:2 * n_act_total + nd],
                            in0=u[:, :, 3], scalar=128.0, in1=u[:, :, 0],
                            op0=Alu.mult, op1=Alu.add)
                    nc.sync.dma_start(y[:, :], y_sb[:])

    nc.compile()
    return nc


def _get_nc(reps=1, **kw):
    key = (reps,) + tuple(sorted(kw.items()))
    if key not in _compiled_nc:
        _compiled_nc[key] = _build_nc(reps, **kw)
    return _compiled_nc[key]


def _to_bf16(a):
    import ml_dtypes

    return np.ascontiguousarray(a).astype(ml_dtypes.bfloat16)


def _host_prep(x, w_lin, b_lin, v, field_idx, dch=1024, xlayout="slab",
               x8=1):
    """Host-side tiny-param preprocessing + sharding. Returns (in_maps, lin)."""
    x = np.asarray(x, dtype=np.float32)
    w_lin = np.asarray(w_lin, dtype=np.float64)
    b_lin = np.asarray(b_lin, dtype=np.float64)
    v = np.asarray(v, dtype=np.float64)
    field_idx = np.asarray(field_idx, dtype=np.int64)

    # Wmat[i, j] = <v[i, field[j]], v[j, field[i]]>
    A = v[:, field_idx, :]                       # [N, N, K]
    Wmat = np.einsum('ijk,jik->ij', A, A)        # [N, N]
    Wu = np.triu(Wmat, 1)
    S = (Wu + Wu.T) * 0.5
    lam, Q = np.linalg.eigh(S)                   # ascending eigenvalues
    # median shift: exactly 128 positive / 128 negative shifted eigenvalues
    c = -(lam[_N // 2 - 1] + lam[_N // 2]) / 2
    mu = lam + c
    pos = np.where(mu > 0)[0]
    neg = np.where(mu <= 0)[0]
    assert len(pos) == _N // 2 and len(neg) == _N // 2
    Qb = Q * np.sqrt(np.abs(mu))[None, :]
    # ACT kind: positive block then negative block (two-segment accum)
    Qact = np.concatenate([Qb[:, pos], Qb[:, neg]], axis=1)
    # DVE kind: pos at even, neg at odd free positions (bn_stats even/odd)
    Qdve = np.empty_like(Qb)
    Qdve[:, 0::2] = Qb[:, pos]
    Qdve[:, 1::2] = Qb[:, neg]
    Qp = np.stack([_to_bf16(Qact).reshape(2, 128, _N),
                   _to_bf16(Qdve).reshape(2, 128, _N)])  # [kind, blk, p, n]

    import ml_dtypes
    nslab = _BS // dch
    x64 = x.astype(np.float64)
    if x8:
        # [core, slab, p, 1, col] per block; block0 bf16, block1 e3m4
        xr = x.reshape(_NCORES, nslab, dch, 2, 128)
        xb = _to_bf16(xr[:, :, :, 0, :]).transpose(0, 1, 3, 2)
        xb = np.ascontiguousarray(xb)[:, :, :, None, :]
        xe = np.ascontiguousarray(xr[:, :, :, 1, :]).astype(
            ml_dtypes.float8_e3m4).transpose(0, 1, 3, 2)
        xe = np.ascontiguousarray(xe)[:, :, :, None, :]
        in_maps = [{"xt": xb[i], "xte": xe[i], "qp": None}
                   for i in range(_NCORES)]
    elif xlayout == "row":
        # [core, blk, p, col] = x^T split into 2 contraction blocks
        xts = x.reshape(_NCORES, _BS, _N).transpose(0, 2, 1)  # [8, N, BS]
        xts = _to_bf16(xts).reshape(_NCORES, 2, 128, _BS)
    else:
        # [core, slab, p, blk, col]: (blk, col) contiguous per partition
        xts = _to_bf16(x).reshape(_NCORES, nslab, dch, 2, 128)
        xts = np.ascontiguousarray(xts.transpose(0, 1, 4, 3, 2))

    if x8:
        for i in range(_NCORES):
            in_maps[i]["qp"] = Qp
    else:
        in_maps = [{"xt": xts[i], "qp": Qp} for i in range(_NCORES)]
    # linear part and the -c||x||^2 spectrum-shift correction, both host-side
    lin = x64 @ w_lin + b_lin[0] - c * np.einsum('bi,bi->b', x64, x64)
    return in_maps, lin


def _y_perm(act_chunks=24, gsz=2):
    """Column permutation: y_sb engine-order col -> chunk id."""
    _, act_order, dve_order = _assign(act_chunks, gsz)
    return np.array(act_order + dve_order)


def _run_device(in_maps, trace=False, reps=1, **kw):
    from concourse.bass_utils import run_bass_kernel_spmd

    nc = _get_nc(reps, **kw)
    res = run_bass_kernel_spmd(
        nc, in_maps, core_ids=list(range(_NCORES)), trace=trace
    )
    _, act_order, dve_order = _assign(kw.get("act_chunks", 18),
                                      kw.get("gsz", 2))
    na = len(act_order)
    perm = np.array(act_order + dve_order)
    inv = np.empty_like(perm)
    inv[perm] = np.arange(len(perm))
    ys = []
    for i in range(_NCORES):
        yr = np.asarray(res.results[i]["y"], dtype=np.float64)
        # ACT chunk k: cols (2k) - (2k+1); DVE chunk j: col 2*na + j
        yc = np.concatenate(
            [yr[:, 0:2 * na:2] - yr[:, 1:2 * na:2], yr[:, 2 * na:]], axis=1)
        ys.append(yc[:, inv].T.reshape(-1))
    yq = np.concatenate(ys)
    return yq, res


def kernel(x, w_lin, b_lin, v, field_idx):
    in_maps, lin = _host_prep(x, w_lin, b_lin, v, field_idx, x8=1)
    yq, _ = _run_device(in_maps, trace=False, x8=1)
    return (lin + yq).astype(np.float32)[:, None]


# revision 28
# speedup vs baseline: 1.1546x; 1.0254x over previous
"""FFM (field-aware factorization machine) forward kernel for 8 TRN2 NeuronCores.

y[b] = x[b] @ w_lin + b_lin + sum_{i<j} Wu[i,j] x[b,i] x[b,j]
with Wu = triu(Wmat, 1), Wmat[i,j] = <v[i, field[j]], v[j, field[i]]>.

Strategy (v6):
  - Host: build Wmat from (v, field_idx) [tiny], symmetrize
    S = (Wu + Wu^T)/2, eigendecompose S = Q diag(lam) Q^T. MEDIAN shift:
    c = -median(lam) makes exactly 128 shifted eigenvalues positive and
    128 negative with |c| tiny (no cancellation amplification), so
      x^T Wu x = sum_n sign(mu_n) (x . q'_n)^2 - c ||x||^2,
    q' = q sqrt(|mu|). The -c||x||^2 correction joins the host linear
    part. The tiny shift keeps quantization error low enough that the
    SECOND contraction block of x ships as fp8e3 (e3m4) - 25% less DMA
    (rel err 0.0138 vs the 0.02 budget, deterministic inputs).
  - Device (data-parallel over batch, 8 cores): per 128-sample chunk,
    PE computes z = x_chunk^T Q' with batch on PSUM partitions and the
    eigen index on the free dim (bf16 block-0 + fp8e3 block-1 matmuls,
    mixed-dtype verified exact). Reduction sum_n sign_n z_n^2 runs on
    the FREE dim, whole PSUM GROUPS per engine:
      * ACT groups: rhs = Q'_act ([pos block | neg block] column order);
        ScalarE Square(psum)+accum_out per 128-col segment -> (pos,neg)
        y-col pair; host subtracts.
      * DVE groups: rhs = Q'_dve (pos at EVEN, neg at ODD columns);
        VectorE bn_stats per chunk straight from PSUM - its even/odd
        (count,mean,count*var) stats give the SIGNED sum in one batched
        5-op fixup:  y = (cv_e + 128 me^2) - (cv_o + 128 mo^2).
        No PSUM->SBUF copy (PSUM read once per DVE instr).
  - x ships pre-transposed in 8 slab DMA pairs on the SP HWDGE ring
    with a deep (8-buf) SBUF pipeline.
  - y columns are engine-ordered (ACT (pos,neg) pairs first, then DVE
    cols); the host subtracts and inverts the permutation.
"""

import numpy as np

_LDW_OPT = {"on": False}


def _install_walrus_patch():
    """Allow flipping walrus --enable-ldw-opt at NEFF-compile time."""
    from concourse import bass_utils
    if getattr(bass_utils, "_ant_ldw_patched", False):
        return
    orig = bass_utils.run_command

    def patched(cmd, *a, **kw):
        if _LDW_OPT["on"] and isinstance(cmd, list):
            cmd = [c.replace("--enable-ldw-opt=false", "--enable-ldw-opt=true")
                   if isinstance(c, str) else c for c in cmd]
        return orig(cmd, *a, **kw)

    bass_utils.run_command = patched
    bass_utils._ant_ldw_patched = True


_B, _N = 65536, 256
_NCORES = 8
_BS = _B // _NCORES   # 8192 samples per core
_NCH = _BS // 128     # 64 batch chunks per core
_DCH = 1024           # DMA slab columns

_compiled_nc = {}


def _assign(act_chunks, gsz, nch=_NCH):
    """Group-level engine assignment: whole groups go to ACT until
    act_chunks is covered (rounded to groups), interleaved evenly.

    Returns (n_act_of[g], act_order, dve_order)."""
    n_groups = nch // gsz
    n_act_groups = min(n_groups, round(act_chunks / gsz))
    # spread ACT groups evenly among all groups
    is_act = [False] * n_groups
    if n_act_groups:
        for i in range(n_act_groups):
            is_act[(i * n_groups) // n_act_groups] = True
    n_act_of = [gsz if a else 0 for a in is_act]
    act_order, dve_order = [], []
    for g in range(n_groups):
        for j in range(gsz):
            c = g * gsz + j
            (act_order if j < n_act_of[g] else dve_order).append(c)
    return n_act_of, act_order, dve_order


def _build_nc(reps=1, mode="full", act_chunks=18, xin_bufs=8,
              gsz=2, pz_bufs=8, dch=1024, dma_alt=False, xlayout="slab",
              ldw_opt=0, salt=0, scr_mode="inplace", fix_eng="vector", obufs=2,
              x8=1):
    """v5 kernel; see module docstring. Probe modes:
      full    - everything
      dmaonly - x DMAs only
      noxdma  - MMs+reducers from undisturbed SBUF (no x DMA)
      nored   - DMAs + MMs, no reducers
      samew   - DMAs + MMs with a CONSTANT stationary operand (junk
                math; probes whether unchanged weights skip/hide LDW)
      samewnx - samew without x DMAs (pure-PE constant-weight probe)
      mmonly  - real MM structure without x DMAs or reducers
    """
    from concourse import bacc, mybir, tile

    f32 = mybir.dt.float32
    bf16 = mybir.dt.bfloat16
    Act = mybir.ActivationFunctionType
    Alu = mybir.AluOpType

    assert _BS % dch == 0
    nslab = _BS // dch
    n_groups = _NCH // gsz
    n_act_of, act_order, dve_order = _assign(act_chunks, gsz)
    n_act_total = len(act_order)
    n_dve_total = len(dve_order)

    _install_walrus_patch()
    _LDW_OPT["on"] = bool(ldw_opt)

    nc = bacc.Bacc("TRN2", target_bir_lowering=False, debug=False)

    # salt defeats the NEFF cache when only compile flags change
    if salt:
        nc.dram_tensor(f"salt{salt}", [1, 1], mybir.dt.float32,
                       kind="ExternalOutput")

    # x pre-transposed. Two DRAM layouts:
    #   row:  [2, 128, _BS] row-major; a slab reads a column range of
    #         every (blk, p) row -> 2*128 descriptors of dch*2B, 16KB
    #         DRAM stride between descriptors.
    #   slab: [nslab, 128, 2, dch]; slab s partition p is contiguous
    #         (2*dch*2B descriptors).
    fp8 = mybir.dt.float8e3
    xte = None
    if x8:
        assert xlayout == "slab"
        xt = nc.dram_tensor("xt", [nslab, 128, 1, dch], bf16,
                            kind="ExternalInput").ap()
        xte = nc.dram_tensor("xte", [nslab, 128, 1, dch], fp8,
                             kind="ExternalInput").ap()
    elif xlayout == "row":
        xt = nc.dram_tensor("xt", [2, 128, _BS], bf16,
                            kind="ExternalInput").ap()
    else:
        xt = nc.dram_tensor("xt", [nslab, 128, 2, dch], bf16,
                            kind="ExternalInput").ap()

    def x_slab_src(s):
        if xlayout == "row":
            return xt[:, :, s * dch:(s + 1) * dch].transpose([1, 0, 2])
        return xt[s]
    qp = nc.dram_tensor("qp", [2, 2, 128, _N], bf16,
                        kind="ExternalInput").ap()
    # y cols: per ACT chunk a (pos, neg) pair, then one col per DVE chunk
    y = nc.dram_tensor("y", [128, _NCH + n_act_total], f32,
                       kind="ExternalOutput").ap()

    dma_engines = [nc.sync, nc.scalar] if dma_alt else [nc.sync]

    with tile.TileContext(nc) as tc:
        with (
            tc.tile_pool(name="const", bufs=1) as cpool,
            tc.tile_pool(name="xin", bufs=xin_bufs) as xpool,
            tc.tile_pool(name="yout", bufs=obufs) as ypool,
            tc.tile_pool(name="stat", bufs=obufs) as stpool,
            tc.tile_pool(name="scr", bufs=4) as spool,
            tc.tile_pool(name="fix", bufs=2) as fpool,
            tc.tile_pool(name="pz", bufs=pz_bufs, space="PSUM") as pzpool,
        ):
            q_sb = cpool.tile([128, 2, 2, _N], bf16)
            nc.sync.dma_start(q_sb[:], qp.transpose([2, 0, 1, 3]))
            xfe = None
            if mode in ("noxdma", "samewnx", "mmonly"):
                if x8:
                    xfix = cpool.tile([128, 1, dch], bf16)
                    nc.sync.dma_start(xfix[:], xt[0])
                    xfe = cpool.tile([128, 1, dch], fp8)
                    nc.sync.dma_start(xfe[:], xte[0])
                else:
                    xfix = cpool.tile([128, 2, dch], bf16)
                    nc.sync.dma_start(xfix[:], x_slab_src(0))

            scr_fix = None
            if scr_mode == "fixed":
                scr_fix = cpool.tile([128, _N], bf16)

            def emit_group(g, pz, y_sb, st_sb, cursors):
                n_act = n_act_of[g]
                for j in range(n_act):
                    for seg in range(2):
                        sl = pz[:, j, seg * 128:(seg + 1) * 128]
                        if scr_mode == "inplace":
                            scr = sl
                        else:
                            scr_t = spool.tile([128, 128], bf16, tag="s")
                            scr = scr_t[:]
                        nc.scalar.activation(
                            scr, sl, Act.Square,
                            accum_out=y_sb[:, cursors[0]:cursors[0] + 1])
                        cursors[0] += 1
                for j in range(n_act, gsz):
                    nc.vector.bn_stats(st_sb[:, cursors[1], :], pz[:, j, :])
                    cursors[1] += 1

            for _rep in range(reps):
                y_sb = ypool.tile([128, _NCH + n_act_total], f32, tag="y")
                st_sb = stpool.tile([128, max(n_dve_total, 1), 6], f32,
                                    tag="st")
                cursors = [0, 0]   # [next ACT y col, next stats slot]
                c = 0
                for s in range(nslab):
                    if mode in ("noxdma", "samewnx", "mmonly"):
                        x_sb = xfix
                        xe_sb = xfe
                    elif x8:
                        x_sb = xpool.tile([128, 1, dch], bf16, tag="x")
                        xe_sb = xpool.tile([128, 1, dch], fp8, tag="xe")
                        eng = dma_engines[s % len(dma_engines)]
                        eng.dma_start(x_sb[:], xt[s])
                        eng.dma_start(xe_sb[:], xte[s])
                    else:
                        x_sb = xpool.tile([128, 2, dch], bf16, tag="x")
                        xe_sb = None
                        eng = dma_engines[s % len(dma_engines)]
                        eng.dma_start(x_sb[:], x_slab_src(s))
                    for k in range(dch // 128):
                        j = c % gsz
                        if j == 0:
                            pz = pzpool.tile([128, gsz, _N], f32, tag="pz")
                        if mode in ("samew", "samewnx"):
                            nc.tensor.matmul(pz[:, j, :],
                                             q_sb[:, 0, 0:128],
                                             x_sb[:, 0, :].rearrange(
                                                 "p (a b) -> p a b", b=256)
                                             [:, k % (dch // 256), :],
                                             start=True, stop=True)
                            c += 1
                            continue
                        kind = 0 if n_act_of[c // gsz] else 1
                        nc.tensor.matmul(pz[:, j, :],
                                         x_sb[:, 0, k * 128:(k + 1) * 128],
                                         q_sb[:, kind, 0, :],
                                         start=True, stop=False)
                        xb1 = xe_sb if x8 else x_sb
                        nc.tensor.matmul(pz[:, j, :],
                                         xb1[:, 0 if x8 else 1,
                                             k * 128:(k + 1) * 128],
                                         q_sb[:, kind, 1, :],
                                         start=False, stop=True)
                        c += 1
                        if mode in ("nored", "samew", "samewnx", "mmonly"):
                            continue
                        if j == gsz - 1:
                            emit_group(c // gsz - 1, pz, y_sb, st_sb, cursors)
                if mode == "full":
                    # fixup: y_dve = cv_e + cv_o + 128*(me^2 + mo^2)
                    nd = n_dve_total
                    if nd:
                        feng = getattr(nc, fix_eng)
                        u = fpool.tile([128, nd, 4], f32, tag="u")
                        feng.tensor_tensor(
                            out=u[:, :, 0], in0=st_sb[:, :, 2],
                            in1=st_sb[:, :, 5], op=Alu.subtract)
                        feng.tensor_tensor(
                            out=u[:, :, 1], in0=st_sb[:, :, 1],
                            in1=st_sb[:, :, 1], op=Alu.mult)
                        feng.tensor_tensor(
                            out=u[:, :, 2], in0=st_sb[:, :, 4],
                            in1=st_sb[:, :, 4], op=Alu.mult)
                        feng.tensor_tensor(
                            out=u[:, :, 3], in0=u[:, :, 1],
                            in1=u[:, :, 2], op=Alu.subtract)
                        nc.vector.scalar_tensor_tensor(
                            out=y_sb[:, 2 * n_act_total:2 * n_act_total + nd],
                            in0=u[:, :, 3], scalar=128.0, in1=u[:, :, 0],
                            op0=Alu.mult, op1=Alu.add)
                    nc.sync.dma_start(y[:, :], y_sb[:])

    nc.compile()
    return nc


def _get_nc(reps=1, **kw):
    key = (reps,) + tuple(sorted(kw.items()))
    if key not in _compiled_nc:
        _compiled_nc[key] = _build_nc(reps, **kw)
    return _compiled_nc[key]


def _to_bf16(a):
    import ml_dtypes

    return np.ascontiguousarray(a).astype(ml_dtypes.bfloat16)


def _host_prep(x, w_lin, b_lin, v, field_idx, dch=1024, xlayout="slab",
               x8=1):
    """Host-side tiny-param preprocessing + sharding. Returns (in_maps, lin)."""
    x = np.asarray(x, dtype=np.float32)
    w_lin = np.asarray(w_lin, dtype=np.float64)
    b_lin = np.asarray(b_lin, dtype=np.float64)
    v = np.asarray(v, dtype=np.float64)
    field_idx = np.asarray(field_idx, dtype=np.int64)

    # Wmat[i, j] = <v[i, field[j]], v[j, field[i]]>
    A = v[:, field_idx, :]                       # [N, N, K]
    Wmat = np.einsum('ijk,jik->ij', A, A)        # [N, N]
    Wu = np.triu(Wmat, 1)
    S = (Wu + Wu.T) * 0.5
    lam, Q = np.linalg.eigh(S)                   # ascending eigenvalues
    # median shift: exactly 128 positive / 128 negative shifted eigenvalues
    c = -(lam[_N // 2 - 1] + lam[_N // 2]) / 2
    mu = lam + c
    pos = np.where(mu > 0)[0]
    neg = np.where(mu <= 0)[0]
    assert len(pos) == _N // 2 and len(neg) == _N // 2
    Qb = Q * np.sqrt(np.abs(mu))[None, :]
    # ACT kind: positive block then negative block (two-segment accum)
    Qact = np.concatenate([Qb[:, pos], Qb[:, neg]], axis=1)
    # DVE kind: pos at even, neg at odd free positions (bn_stats even/odd)
    Qdve = np.empty_like(Qb)
    Qdve[:, 0::2] = Qb[:, pos]
    Qdve[:, 1::2] = Qb[:, neg]
    Qp = np.stack([_to_bf16(Qact).reshape(2, 128, _N),
                   _to_bf16(Qdve).reshape(2, 128, _N)])  # [kind, blk, p, n]

    import ml_dtypes
    nslab = _BS // dch
    x64 = x.astype(np.float64)
    if x8:
        # [core, slab, p, 1, col] per block; block0 bf16, block1 e3m4
        xr = x.reshape(_NCORES, nslab, dch, 2, 128)
        xb = _to_bf16(xr[:, :, :, 0, :]).transpose(0, 1, 3, 2)
        xb = np.ascontiguousarray(xb)[:, :, :, None, :]
        xe = np.ascontiguousarray(xr[:, :, :, 1, :]).astype(
            ml_dtypes.float8_e3m4).transpose(0, 1, 3, 2)
        xe = np.ascontiguousarray(xe)[:, :, :, None, :]
        in_maps = [{"xt": xb[i], "xte": xe[i], "qp": None}
                   for i in range(_NCORES)]
    elif xlayout == "row":
        # [core, blk, p, col] = x^T split into 2 contraction blocks
        xts = x.reshape(_NCORES, _BS, _N).transpose(0, 2, 1)  # [8, N, BS]
        xts = _to_bf16(xts).reshape(_NCORES, 2, 128, _BS)
    else:
        # [core, slab, p, blk, col]: (blk, col) contiguous per partition
        xts = _to_bf16(x).reshape(_NCORES, nslab, dch, 2, 128)
        xts = np.ascontiguousarray(xts.transpose(0, 1, 4, 3, 2))

    if x8:
        for i in range(_NCORES):
            in_maps[i]["qp"] = Qp
    else:
        in_maps = [{"xt": xts[i], "qp": Qp} for i in range(_NCORES)]
    # linear part and the -c||x||^2 spectrum-shift correction, both host-side
    lin = x64 @ w_lin + b_lin[0] - c * np.einsum('bi,bi->b', x64, x64)
    return in_maps, lin


def _y_perm(act_chunks=24, gsz=2):
    """Column permutation: y_sb engine-order col -> chunk id."""
    _, act_order, dve_order = _assign(act_chunks, gsz)
    return np.array(act_order + dve_order)


def _run_device(in_maps, trace=False, reps=1, **kw):
    from concourse.bass_utils import run_bass_kernel_spmd

    nc = _get_nc(reps, **kw)
    res = run_bass_kernel_spmd(
        nc, in_maps, core_ids=list(range(_NCORES)), trace=trace
    )
    _, act_order, dve_order = _assign(kw.get("act_chunks", 18),
                                      kw.get("gsz", 2))
    na = len(act_order)
    perm = np.array(act_order + dve_order)
    inv = np.empty_like(perm)
    inv[perm] = np.arange(len(perm))
    ys = []
    for i in range(_NCORES):
        yr = np.asarray(res.results[i]["y"], dtype=np.float64)
        # ACT chunk k: cols (2k) - (2k+1); DVE chunk j: col 2*na + j
        yc = np.concatenate(
            [yr[:, 0:2 * na:2] - yr[:, 1:2 * na:2], yr[:, 2 * na:]], axis=1)
        ys.append(yc[:, inv].T.reshape(-1))
    yq = np.concatenate(ys)
    return yq, res


def kernel(x, w_lin, b_lin, v, field_idx):
    in_maps, lin = _host_prep(x, w_lin, b_lin, v, field_idx, x8=1)
    yq, _ = _run_device(in_maps, trace=False, x8=1)
    return (lin + yq).astype(np.float32)[:, None]
